# revision 44
# baseline (speedup 1.0000x reference)
"""GraphTransformer 2-layer (TransformerConv x2) on 8 Trainium2 NeuronCores.

Sharding: destination-node partitioning with degree-sorted padded tiles.
  - Pad N=50000 -> N'=50176 (392 tiles of 128 nodes). Sort nodes by
    in-degree, bin-pack the tiles onto 8 cores (49 each, balancing slots).
  - Each core receives x^T (fp16) in a per-core *rotated* node order (its
    own nodes first) and computes the full layer-1 K|V table [N',512] fp16
    on-device (PSUM evictions fanned out across DVE/ACT), plus Q|S for its
    own nodes.
  - Attention processes dst-tiles in degree-padded PAIRS: per neighbor
    rank one indirect DMA (single index per partition - the only gather
    shape TRN2 hardware supports) fetches that rank's kv rows for all 128
    dst nodes, using indices pre-scaled into a flat view of the table.
    Padding slots point at row 0; their logits get a -30000 bias so
    exp() == 0. Logits carry a constant -8 shift (folded into the bias)
    so exp stays in fp16 range without a per-row max pass.
  - V features are stored (c,h)-major so the alpha-broadcast multiply has
    a packed innermost dim (fp16 2x DVE mode); segmented reductions are
    in-place halving tree-adds on DVE; softmax normalization uses the
    fp32 reciprocal (fp16 would overflow for near-empty rows).
  - Layer-2 K|V [N',20] fp16 is computed from the local h chunk and
    AllGathered per 8-tile group (group-major layout, overlapped under
    the layer-1 attention); layer-2 attention math runs fully grouped
    (8 tiles per instruction, degree-padded to the group max).
All indices/degrees/tile shapes are baked in at build time from the actual
inputs. kernel() builds + runs the single-launch SPMD program and
unpermutes the output on the host.
"""

import numpy as np

N_CORES = 8
N = 50000
IN_DIM = 128
D1 = 256            # heads*hid layer1
H1, C1 = 8, 32
D2 = 10             # layer2 out channels (1 head)
P = 128
NEG = -30000.0      # kills a slot in fp16: exp(logit + NEG) == 0
SHIFT = -8.0        # constant logit shift, keeps exp() in fp16 range
GRP = 8             # tiles per metadata DMA group (49 = 6*8 + 1)
DEBUG_H = False


def _pairs(per_core):
    ps, j = [], 0
    while j < per_core:
        t = 2 if j + 1 < per_core else 1
        ps.append((j, t))
        j += t
    return ps


def _align_pairs(Ds_pos):
    d = list(Ds_pos)
    for j0, t in _pairs(len(d)):
        if t == 2:
            d[j0] = d[j0 + 1] = max(d[j0], d[j0 + 1])
    return d


def _plan(edge_index):
    src = np.asarray(edge_index[0], dtype=np.int64)
    dst = np.asarray(edge_index[1], dtype=np.int64)
    deg = np.bincount(dst, minlength=N)
    NP_ = ((N + N_CORES * P - 1) // (N_CORES * P)) * (N_CORES * P)  # 50176
    n_tiles = NP_ // P                                              # 392
    per_core = n_tiles // N_CORES                                   # 49

    degp = np.concatenate([deg, np.zeros(NP_ - N, np.int64)])
    order0 = np.argsort(degp, kind="stable")        # old(padded) ids, deg asc
    tile_of = order0.reshape(n_tiles, P)            # prelim tile -> old ids
    tile_D = degp[tile_of].max(axis=1)

    # SPMD cost is sum_j max_c D[c, j]: deal degree-sorted tiles
    # round-robin so per-position degree spreads are minimal
    t_order = np.argsort(tile_D, kind="stable")
    assign = [[] for _ in range(N_CORES)]
    for r, t in enumerate(t_order):
        assign[r % N_CORES].append(int(t))

    final_tiles = [t for c in range(N_CORES) for t in assign[c]]
    perm = tile_of[final_tiles].reshape(-1)         # new id -> old(padded) id
    inv = np.empty(NP_, np.int64)
    inv[perm] = np.arange(NP_)

    Ds = degp[perm].reshape(n_tiles, P).max(axis=1).astype(np.int64)

    # per-(new)tile neighbor tables in NEW ids; pad idx=0, bias=NEG
    dst_new = inv[dst]
    src_new = inv[src]
    eo = np.argsort(dst_new, kind="stable")
    dst_s = dst_new[eo]
    src_s = src_new[eo]
    row_start = np.searchsorted(dst_s, np.arange(NP_))
    row_end = np.searchsorted(dst_s, np.arange(NP_) + 1)

    idx_tiles, bias_tiles = [], []
    for t in range(n_tiles):
        D = int(Ds[t])
        it = np.zeros((P, D), np.int64)
        bt = np.full((P, D), NEG, np.float32)
        for p in range(P):
            s, e = row_start[t * P + p], row_end[t * P + p]
            k = e - s
            it[p, :k] = src_s[s:e]
            bt[p, :k] = 0.0
        idx_tiles.append(it)
        bias_tiles.append(bt)

    return dict(NP=NP_, n_tiles=n_tiles, per_core=per_core, perm=perm,
                inv=inv, Ds=[int(d) for d in Ds], idx_tiles=idx_tiles,
                bias_tiles=bias_tiles)


def _groups(per_core):
    gs = []
    j = 0
    while j < per_core:
        gs.append(list(range(j, min(j + GRP, per_core))))
        j += GRP
    return gs


def _build_program(NP_, per_core, Ds_pos, sim1=False):
    import concourse.bass as bass
    import concourse.mybir as mybir
    from concourse import bacc
    from concourse.tile import TileContext
    from concourse.masks import make_identity

    f32 = mybir.dt.float32
    f16 = mybir.dt.float16
    i32 = mybir.dt.int32
    NOWN = per_core * P
    slots1 = sum(P * d for d in Ds_pos)
    Dmax = max(Ds_pos)
    groups = _groups(per_core)
    Dg = [max(Ds_pos[j] for j in g) for g in groups]   # group-padded degree
    slots2 = sum(P * len(g) * Dg[gi] for gi, g in enumerate(groups))
    # group offsets into the (group-major packed) layer-1 meta blobs
    goff1 = [0]
    for g in groups:
        goff1.append(goff1[-1] + P * sum(Ds_pos[j] for j in g))
    goff2 = [0]
    for gi, g in enumerate(groups):
        goff2.append(goff2[-1] + P * len(g) * Dg[gi])
    # group-major AllGather layout: [group][core][tile][p][f]
    gbase = [0]
    for g in groups:
        gbase.append(gbase[-1] + N_CORES * len(g) * P * 2 * D2)

    nc = bacc.Bacc("TRN2", target_bir_lowering=False, debug=False,
                   num_devices=1 if sim1 else N_CORES)

    xT = nc.dram_tensor("xT", [IN_DIM, NP_], f16, kind="ExternalInput")
    w_kv1 = nc.dram_tensor("w_kv1", [IN_DIM, 2 * D1], f16, kind="ExternalInput")
    w_qs1 = nc.dram_tensor("w_qs1", [IN_DIM, 2 * D1], f16, kind="ExternalInput")
    w_2 = nc.dram_tensor("w_2", [D1, 4 * D2], f16, kind="ExternalInput")
    idx1_f = nc.dram_tensor("idx1_f", [slots1], i32, kind="ExternalInput")
    idx2_f = nc.dram_tensor("idx2_f", [slots2], i32, kind="ExternalInput")
    bias1_f = nc.dram_tensor("bias1_f", [slots1], f16, kind="ExternalInput")
    bias2_f = nc.dram_tensor("bias2_f", [slots2], f16, kind="ExternalInput")
    out_d = nc.dram_tensor("out", [NOWN, D2], f32, kind="ExternalOutput")
    dbg_h = nc.dram_tensor("dbg_h", [NOWN, D1], f16,
                           kind="ExternalOutput") if DEBUG_H else None
    dbg_al = nc.dram_tensor("dbg_al", [slots1 * H1], f16,
                            kind="ExternalOutput") if DEBUG_H else None

    kv1_t = nc.dram_tensor("kv1_t", [1, NP_ * 2 * D1], f16)   # flat
    qs1_t = nc.dram_tensor("qs1_t", [NOWN, 2 * D1], f16)
    kv2_own = nc.dram_tensor("kv2_own", [1, NOWN * 2 * D2], f16)
    kv2_full = nc.dram_tensor("kv2_full", [1, NP_ * 2 * D2], f16,  # flat
                              addr_space="Shared")

    X = mybir.AxisListType.X
    MUL = mybir.AluOpType.mult
    ADD = mybir.AluOpType.add
    MAX = mybir.AluOpType.max
    EXP = mybir.ActivationFunctionType.Exp

    with TileContext(nc) as tc:
        with tc.tile_pool(name="wpool", bufs=1) as wpool:
            w_kv1_s = wpool.tile([IN_DIM, 2 * D1], f16)
            nc.sync.dma_start(out=w_kv1_s[:], in_=w_kv1[:, :])
            w_qs1_s = wpool.tile([IN_DIM, 2 * D1], f16)
            nc.sync.dma_start(out=w_qs1_s[:], in_=w_qs1[:, :])
            # layer-2 weights: rows 0:128 -> cols 0:40, rows 128:256 -> 40:80
            w2_s = wpool.tile([P, 2 * 4 * D2], f16)
            nc.sync.dma_start(out=w2_s[:, 0:4 * D2], in_=w_2[0:P, :])
            nc.sync.dma_start(out=w2_s[:, 4 * D2:8 * D2], in_=w_2[P:2 * P, :])
            ident = wpool.tile([P, P], f16)
            make_identity(nc, ident[:])
            # layer-2 Q|S per own tile, resident in SBUF
            qs2_res = wpool.tile([P, per_core * 2 * D2], f16)

            # ================= P1: layer-1 projections =================
            WB = 8           # kv output tiles staged per DRAM write
            XB = P * WB
            evict = [lambda out, in_: nc.vector.tensor_copy(out=out, in_=in_),
                     nc.scalar.copy,
                     lambda out, in_: nc.vector.tensor_copy(out=out, in_=in_),
                     nc.scalar.copy]
            ev = 0
            with tc.tile_pool(name="p1x", bufs=8) as p1x, \
                 tc.tile_pool(name="p1ps", bufs=8, space="PSUM") as p1ps, \
                 tc.tile_pool(name="p1o", bufs=8) as p1o:
                for blk in range(NP_ // XB):
                    xT_s = p1x.tile([P, XB], f16, tag="xT")
                    nc.sync.dma_start(out=xT_s[:],
                                      in_=xT[:, blk * XB:(blk + 1) * XB])
                    kv_o = p1o.tile([P, WB * 2 * D1], f16, tag="kv")
                    nown = min(max(per_core - blk * WB, 0), WB)
                    qs_o = None
                    if nown:
                        qs_o = p1o.tile([P, WB * 2 * D1], f16, tag="qso")
                    for jj in range(WB):
                        t = blk * WB + jj
                        lhsT = xT_s[:, jj * P:(jj + 1) * P]
                        ps = p1ps.tile([P, 2 * D1], f32, tag="ps")
                        nc.tensor.matmul(out=ps[:], lhsT=lhsT, rhs=w_kv1_s[:],
                                         start=True, stop=True)
                        evict[ev % 4](
                            out=kv_o[:, jj * 2 * D1:(jj + 1) * 2 * D1],
                            in_=ps[:])
                        ev += 1
                        if t < per_core:   # own nodes (rotated order)
                            ps2 = p1ps.tile([P, 2 * D1], f32, tag="ps")
                            nc.tensor.matmul(out=ps2[:], lhsT=lhsT,
                                             rhs=w_qs1_s[:],
                                             start=True, stop=True)
                            evict[ev % 4](
                                out=qs_o[:, jj * 2 * D1:(jj + 1) * 2 * D1],
                                in_=ps2[:])
                            ev += 1
                    if nown:
                        nc.sync.dma_start(
                            out=qs1_t[blk * XB:blk * XB + nown * P, :]
                                .rearrange("(j p) f -> p j f", p=P),
                            in_=qs_o[:, 0:nown * 2 * D1]
                                .rearrange("p (j f) -> p j f", j=nown))
                    HB = WB // 2
                    for hb in range(2):
                        base = (blk * XB + hb * HB * P) * 2 * D1
                        nc.sync.dma_start(
                            out=kv1_t[0:1, base:base + HB * P * 2 * D1]
                                .rearrange("x (j p f) -> p (x j) f",
                                           p=P, j=HB),
                            in_=kv_o[:, hb * HB * 2 * D1:
                                     (hb + 1) * HB * 2 * D1]
                                .rearrange("p (j f) -> p j f", j=HB))
            del evict

            # ========== P2+P3: layer-1 attention + layer-2 projections ==========
            PDmax = max(t * Ds_pos[j0] for j0, t in _pairs(per_core))
            meta2_pool = tc.tile_pool(name="meta2", bufs=7)
            meta2 = meta2_pool.__enter__()
            idx2_tiles, bias2_tiles = [], []
            for gi, g in enumerate(groups):
                GS = len(g) * Dg[gi]
                idx_g2 = meta2.tile([P, GRP * Dmax], i32, tag="idx2")
                nc.sync.dma_start(
                    out=idx_g2[:, 0:GS],
                    in_=idx2_f[goff2[gi]:goff2[gi] + P * GS]
                        .rearrange("(p d) -> p d", d=GS))
                bias_g2 = meta2.tile([P, GRP * Dmax], f16, tag="bias2")
                nc.sync.dma_start(
                    out=bias_g2[:, 0:GS],
                    in_=bias2_f[goff2[gi]:goff2[gi] + P * GS]
                        .rearrange("(p d) -> p d", d=GS))
                idx2_tiles.append(idx_g2)
                bias2_tiles.append(bias_g2)
            with tc.tile_pool(name="kvb", bufs=3) as kvb, \
                 tc.tile_pool(name="qsp", bufs=2) as qsp, \
                 tc.tile_pool(name="meta", bufs=3) as meta, \
                 tc.tile_pool(name="small", bufs=3) as small, \
                 tc.tile_pool(name="hps", bufs=3, space="PSUM") as hps, \
                 tc.tile_pool(name="houtp", bufs=3) as houtp, \
                 tc.tile_pool(name="stg", bufs=2) as stg:
                for gi, g in enumerate(groups):
                    g0 = g[0]
                    GD = sum(Ds_pos[j] for j in g)
                    qs_g = qsp.tile([P, GRP * 2 * D1], f16, tag="qs")
                    nc.sync.dma_start(
                        out=qs_g[:, 0:len(g) * 2 * D1]
                            .rearrange("p (j f) -> p j f", j=len(g)),
                        in_=qs1_t[g0 * P:(g0 + len(g)) * P, :]
                            .rearrange("(j p) f -> p j f", p=P))
                    idx_g = meta.tile([P, GRP * Dmax], i32, tag="idx")
                    nc.sync.dma_start(
                        out=idx_g[:, 0:GD],
                        in_=idx1_f[goff1[gi]:goff1[gi] + P * GD]
                            .rearrange("(p d) -> p d", d=GD))
                    bias_g = meta.tile([P, GRP * Dmax], f16, tag="bias")
                    nc.sync.dma_start(
                        out=bias_g[:, 0:GD],
                        in_=bias1_f[goff1[gi]:goff1[gi] + P * GD]
                            .rearrange("(p d) -> p d", d=GD))
                    kv2_stg = stg.tile([P, GRP * 2 * D2], f16, tag="kv2s")
                    toff = 0
                    jl = 0
                    while jl < len(g):
                        T = 2 if jl + 1 < len(g) else 1
                        j = g0 + jl
                        D = Ds_pos[j]
                        E = T * D
                        idx_s = idx_g[:, toff:toff + E]
                        bias_s = bias_g[:, toff:toff + E]
                        toff += E
                        qsv = qs_g[:, jl * 2 * D1:(jl + T) * 2 * D1] \
                            .rearrange("p (t f) -> p t f", t=T)
                        kv_s = kvb.tile([P, PDmax * 2 * D1], f16, tag="kv")
                        for col in range(E):
                            nc.gpsimd.indirect_dma_start(
                                out=kv_s[:, col * 2 * D1:(col + 1) * 2 * D1],
                                out_offset=None,
                                in_=kv1_t[:, :],
                                in_offset=bass.IndirectOffsetOnAxis(
                                    ap=idx_s[:, col:col + 1], axis=1))
                        kv4 = kv_s[:, 0:E * 2 * D1].rearrange(
                            "p (t d f) -> p t d f", t=T, d=D)
                        kve = kv_s[:, 0:E * 2 * D1].rearrange(
                            "p (e f) -> p e f", e=E)
                        # q (dot) k -> in-place over the gathered k half
                        nc.vector.tensor_tensor(
                            out=kv4[:, :, :, 0:D1], in0=kv4[:, :, :, 0:D1],
                            in1=qsv[:, :, 0:D1].unsqueeze(2)
                                .to_broadcast([P, T, D, D1]),
                            op=MUL)
                        # sum over c: in-place halving tree on [P, E, H, C]
                        kvh = kve[:, :, 0:D1].rearrange(
                            "p e (h c) -> p e h c", h=H1)
                        w = C1
                        while w > 1:
                            half = w // 2
                            nc.vector.tensor_tensor(
                                out=kvh[:, :, :, 0:half],
                                in0=kvh[:, :, :, 0:half],
                                in1=kvh[:, :, :, w - half:w],
                                op=ADD)
                            w = w - half
                        lgv = kvh[:, :, :, 0:1]          # [P, E, H, 1] strided
                        nc.vector.tensor_tensor(
                            out=lgv, in0=lgv,
                            in1=bias_s.unsqueeze(2).unsqueeze(3)
                                .to_broadcast([P, E, H1, 1]),
                            op=ADD)
                        # exp (ACT) compacts strided logits -> packed alpha
                        al = small.tile([P, PDmax * H1], f16, tag="al")
                        nc.scalar.activation(
                            out=al[:, 0:E * H1].rearrange(
                                "p (e h c) -> p e h c", h=H1, c=1),
                            in_=lgv, func=EXP)
                        sm = small.tile([P, 2 * H1], f32, tag="sm")
                        nc.vector.reduce_sum(
                            out=sm[:, 0:T * H1].rearrange(
                                "p (t h) -> p t h", t=T),
                            in_=al[:, 0:E * H1].rearrange(
                                "p (t d h) -> p t h d", t=T, h=H1),
                            axis=X)
                        nc.vector.tensor_scalar_add(
                            out=sm[:, 0:T * H1], in0=sm[:, 0:T * H1],
                            scalar1=1e-16)
                        rc = small.tile([P, 2 * H1], f32, tag="rc")
                        nc.vector.reciprocal(out=rc[:, 0:T * H1],
                                             in_=sm[:, 0:T * H1])
                        # normalize alpha (fp32 rc: avoids f16 inf for
                        # near-empty rows), then alpha * v split DVE / Pool
                        nc.vector.tensor_tensor(
                            out=al[:, 0:E * H1].rearrange(
                                "p (t d h) -> p t d h", t=T, h=H1),
                            in0=al[:, 0:E * H1].rearrange(
                                "p (t d h) -> p t d h", t=T, h=H1),
                            in1=rc[:, 0:T * H1].rearrange(
                                "p (t h) -> p t h", t=T).unsqueeze(2)
                                .to_broadcast([P, T, D, H1]),
                            op=MUL)
                        alq = al[:, 0:E * H1].rearrange(
                            "p (t d h) -> p t d h", t=T, h=H1)
                        CSPL = 32  # c-range 0:CSPL on DVE, rest on Pool
                        h_s = houtp.tile([P, 2 * D1], f16, tag="h")
                        zmin = houtp.tile([P, 2 * D1], f16, tag="zmin")
                        for tt in range(T):
                            alt = alq[:, tt, :, :]
                            vvt = kv4[:, tt, :, D1:2 * D1]
                            if CSPL < C1:
                                nc.gpsimd.tensor_tensor(
                                    out=vvt[:, :, CSPL * H1:D1].rearrange(
                                        "p d (c h) -> p d c h", h=H1),
                                    in0=vvt[:, :, CSPL * H1:D1].rearrange(
                                        "p d (c h) -> p d c h", h=H1),
                                    in1=alt.unsqueeze(2).to_broadcast(
                                        [P, D, C1 - CSPL, H1]),
                                    op=MUL)
                            nc.vector.tensor_tensor(
                                out=vvt[:, :, 0:CSPL * H1].rearrange(
                                    "p d (c h) -> p d c h", h=H1),
                                in0=vvt[:, :, 0:CSPL * H1].rearrange(
                                    "p d (c h) -> p d c h", h=H1),
                                in1=alt.unsqueeze(2).to_broadcast(
                                    [P, D, CSPL, H1]),
                                op=MUL)
                            # sum over d: in-place halving tree [P, D, 256]
                            w = D
                            while w > 1:
                                half = w // 2
                                nc.vector.tensor_tensor(
                                    out=vvt[:, 0:half, :],
                                    in0=vvt[:, 0:half, :],
                                    in1=vvt[:, w - half:w, :],
                                    op=ADD)
                                w = w - half
                            hh = h_s[:, tt * D1:(tt + 1) * D1]
                            zz = zmin[:, tt * D1:(tt + 1) * D1]
                            # skip connection: h = attsum + x @ Ws
                            nc.vector.tensor_add(out=hh, in0=vvt[:, 0, :],
                                                 in1=qsv[:, tt, D1:2 * D1])
                            # ELU: h = max(z, min(exp(z) - 1, 0))
                            nc.scalar.activation(out=zz, in_=hh, func=EXP)
                            nc.vector.tensor_scalar(
                                out=zz, in0=zz, scalar1=-1.0, scalar2=0.0,
                                op0=ADD, op1=mybir.AluOpType.min)
                            nc.vector.tensor_tensor(out=hh, in0=hh,
                                                    in1=zz, op=MAX)

                        if DEBUG_H:
                            nc.sync.dma_start(
                                out=dbg_al[(goff1[gi] + (toff - E) * P) * H1:
                                           (goff1[gi] + toff * P) * H1]
                                    .rearrange("(p e) -> p e", p=P),
                                in_=al[:, 0:E * H1])
                            nc.sync.dma_start(
                                out=dbg_h[j * P:(j + T) * P, :]
                                    .rearrange("(t p) f -> p t f", p=P),
                                in_=h_s[:, 0:T * D1]
                                    .rearrange("p (t f) -> p t f", t=T))
                        # ---- layer-2 projections ----
                        for tt in range(T):
                            hh = h_s[:, tt * D1:(tt + 1) * D1]
                            hT0 = hps.tile([P, P], f16, tag="hT")
                            nc.tensor.transpose(out=hT0[:], in_=hh[:, 0:P],
                                                identity=ident[:])
                            hT0s = houtp.tile([P, P], f16, tag="hT0s")
                            nc.scalar.copy(out=hT0s[:], in_=hT0[:])
                            hT1 = hps.tile([P, P], f16, tag="hT")
                            nc.tensor.transpose(out=hT1[:], in_=hh[:, P:2 * P],
                                                identity=ident[:])
                            hT1s = houtp.tile([P, P], f16, tag="hT1s")
                            nc.scalar.copy(out=hT1s[:], in_=hT1[:])
                            ps = hps.tile([P, 4 * D2], f32, tag="ps2")
                            nc.tensor.matmul(out=ps[:], lhsT=hT0s[:],
                                             rhs=w2_s[:, 0:4 * D2],
                                             start=True, stop=False)
                            nc.tensor.matmul(out=ps[:], lhsT=hT1s[:],
                                             rhs=w2_s[:, 4 * D2:8 * D2],
                                             start=False, stop=True)
                            nc.scalar.copy(
                                out=kv2_stg[:, (jl + tt) * 2 * D2:
                                            (jl + tt + 1) * 2 * D2],
                                in_=ps[:, 0:2 * D2])
                            nc.scalar.copy(
                                out=qs2_res[:, (j + tt) * 2 * D2:
                                            (j + tt + 1) * 2 * D2],
                                in_=ps[:, 2 * D2:4 * D2])
                        jl += T
                    nc.sync.dma_start(
                        out=kv2_own[0:1, g0 * P * 2 * D2:
                                    (g0 + len(g)) * P * 2 * D2]
                            .rearrange("x (j p f) -> p (x j) f",
                                       p=P, j=len(g)),
                        in_=kv2_stg[:, 0:len(g) * 2 * D2]
                            .rearrange("p (j f) -> p j f", j=len(g)))
                    # AllGather this group's kv2 chunk (overlaps with P2)
                    lo = g0 * P * 2 * D2
                    ln = len(g) * P * 2 * D2
                    if sim1:
                        for c in range(N_CORES):
                            nc.scalar.dma_start(
                                out=kv2_full[0:1, gbase[gi] + c * ln:
                                             gbase[gi] + (c + 1) * ln],
                                in_=kv2_own[0:1, lo:lo + ln])
                    else:
                        nc.gpsimd.collective_compute(
                            "AllGather", mybir.AluOpType.bypass,
                            replica_groups=[list(range(N_CORES))],
                            ins=[kv2_own.ap()[0:1, lo:lo + ln].opt()],
                            outs=[kv2_full.ap()
                                  [0:1, gbase[gi]:gbase[gi + 1]].opt()],
                        )

            # ============ P5: layer-2 attention (fully grouped) ============
            with tc.tile_pool(name="kvb2", bufs=7) as kvb2, \
                 tc.tile_pool(name="small2", bufs=3) as small2, \
                 tc.tile_pool(name="outp", bufs=3) as outp:
                for gi, g in enumerate(groups):
                    g0 = g[0]
                    L = len(g)
                    Dgi = Dg[gi]
                    GS = L * Dgi                     # padded slots per group
                    idx_g = idx2_tiles[gi]
                    bias_g = bias2_tiles[gi]
                    kv_s = kvb2.tile([P, GRP * Dmax * 2 * D2], f16, tag="kv2")
                    for col in range(GS):
                        nc.gpsimd.indirect_dma_start(
                            out=kv_s[:, col * 2 * D2:(col + 1) * 2 * D2],
                            out_offset=None,
                            in_=kv2_full[:, :],
                            in_offset=bass.IndirectOffsetOnAxis(
                                ap=idx_g[:, col:col + 1], axis=1))
                    kv4 = kv_s[:, 0:GS * 2 * D2].rearrange(
                        "p (l d f) -> p l d f", l=L, d=Dgi)
                    qs4 = qs2_res[:, g0 * 2 * D2:(g0 + L) * 2 * D2].rearrange(
                        "p (l f) -> p l f", l=L)
                    # q (dot) k
                    nc.vector.tensor_tensor(
                        out=kv4[:, :, :, 0:D2], in0=kv4[:, :, :, 0:D2],
                        in1=qs4[:, :, 0:D2].unsqueeze(2)
                            .to_broadcast([P, L, Dgi, D2]),
                        op=MUL)
                    lg = small2.tile([P, GRP * Dmax], f32, tag="lg2")
                    lgv = lg[:, 0:GS].rearrange("p (l d) -> p l d", l=L)
                    nc.vector.reduce_sum(out=lgv, in_=kv4[:, :, :, 0:D2],
                                         axis=X)
                    nc.vector.tensor_add(out=lg[:, 0:GS], in0=lg[:, 0:GS],
                                         in1=bias_g[:, 0:GS])
                    nc.scalar.activation(out=lg[:, 0:GS], in_=lg[:, 0:GS],
                                         func=EXP)
                    sm = small2.tile([P, GRP], f32, tag="sm2")
                    nc.vector.reduce_sum(out=sm[:, 0:L], in_=lgv, axis=X)
                    nc.vector.tensor_scalar_add(out=sm[:, 0:L], in0=sm[:, 0:L],
                                                scalar1=1e-16)
                    rc = small2.tile([P, GRP], f32, tag="rc2")
                    nc.vector.reciprocal(out=rc[:, 0:L], in_=sm[:, 0:L])
                    # alpha * v
                    av2 = small2.tile([P, GRP * Dmax * D2], f32, tag="av2")
                    av4 = av2[:, 0:GS * D2].rearrange(
                        "p (l d f) -> p l d f", l=L, d=Dgi)
                    nc.vector.tensor_tensor(
                        out=av4,
                        in0=kv4[:, :, :, D2:2 * D2],
                        in1=lgv.unsqueeze(3).to_broadcast([P, L, Dgi, D2]),
                        op=MUL)
                    out_stg = outp.tile([P, GRP * D2], f32, tag="outs")
                    ov = out_stg[:, 0:L * D2].rearrange(
                        "p (l f) -> p l f", l=L)
                    nc.vector.reduce_sum(
                        out=ov,
                        in_=av4.transpose([0, 1, 3, 2]),
                        axis=X)
                    nc.vector.tensor_tensor(
                        out=ov, in0=ov,
                        in1=rc[:, 0:L].unsqueeze(2).to_broadcast([P, L, D2]),
                        op=MUL)
                    nc.vector.tensor_tensor(
                        out=ov, in0=ov, in1=qs4[:, :, D2:2 * D2], op=ADD)
                    nc.sync.dma_start(
                        out=out_d[g0 * P:(g0 + L) * P, :]
                            .rearrange("(j p) f -> p j f", p=P),
                        in_=out_stg[:, 0:L * D2]
                            .rearrange("p (j f) -> p j f", j=L))
            meta2_pool.__exit__(None, None, None)

    nc.compile()
    return nc


_CACHE = {}


def _get_program(NP_, per_core, Ds_pos):
    key = (NP_, per_core, tuple(Ds_pos))
    if key not in _CACHE:
        _CACHE[key] = _build_program(NP_, per_core, Ds_pos)
    return _CACHE[key]


def _vperm():
    # (h,c)-major -> (c,h)-major column permutation for V / skip features
    return np.arange(D1).reshape(H1, C1).T.reshape(-1)


def _prepare(inputs):
    x = np.asarray(inputs["x"], np.float32)
    edge_index = np.asarray(inputs["edge_index"])
    plan = _plan(edge_index)
    NP_ = plan["NP"]
    per_core = plan["per_core"]
    Ds = plan["Ds"]
    NOWN = per_core * P

    # position-aligned degrees (SPMD: one program for all cores)
    Ds_pos = _align_pairs(
        [max(Ds[c * per_core + j] for c in range(N_CORES))
         for j in range(per_core)])

    s1 = 1.0 / np.sqrt(np.float32(C1))
    s2 = 1.0 / np.sqrt(np.float32(D2))
    vp = _vperm()
    w1v = np.asarray(inputs["w1v"], np.float32)[:, vp]     # (c,h)-major
    w1s = np.asarray(inputs["w1s"], np.float32)[:, vp]
    w_kv1 = np.ascontiguousarray(
        np.concatenate([inputs["w1k"], w1v], axis=1), np.float16)
    w_qs1 = np.ascontiguousarray(
        np.concatenate([np.asarray(inputs["w1q"]) * s1, w1s], axis=1),
        np.float16)
    # layer-2 weights take (c,h)-major h rows; pack [k|v|q|s] -> [256, 40]
    w2 = np.concatenate(
        [np.asarray(inputs["w2k"], np.float32),
         np.asarray(inputs["w2v"], np.float32),
         np.asarray(inputs["w2q"], np.float32) * s2,
         np.asarray(inputs["w2s"], np.float32)], axis=1)[vp]
    w2 = np.ascontiguousarray(w2, np.float16)
    # biases are all zero for this problem; assert to be safe
    for bn in ("b1q", "b1k", "b1v", "b1s", "b2q", "b2k", "b2v", "b2s"):
        assert not np.any(np.asarray(inputs[bn])), f"nonzero bias {bn}"

    nc = _get_program(NP_, per_core, Ds_pos)

    xpad = np.concatenate([x, np.zeros((NP_ - N, IN_DIM), np.float32)])
    x_new = xpad[plan["perm"]]
    xT_new = np.ascontiguousarray(x_new.T.astype(np.float16))

    groups = _groups(per_core)
    Dg = [max(Ds_pos[j] for j in g) for g in groups]
    glen = np.array([len(g) for g in groups])
    gbase = np.zeros(len(groups) + 1, np.int64)
    for gi, g in enumerate(groups):
        gbase[gi + 1] = gbase[gi] + N_CORES * len(g) * P * 2 * D2

    in_maps = []
    for c in range(N_CORES):
        own0 = c * NOWN
        rot = np.concatenate([np.arange(own0, own0 + NOWN),
                              np.arange(0, own0),
                              np.arange(own0 + NOWN, NP_)])
        inv_rot = np.empty(NP_, np.int64)
        inv_rot[rot] = np.arange(NP_)
        xT_c = np.ascontiguousarray(xT_new[:, rot])
        idx1_list, idx2_list, b1_list, b2_list = [], [], [], []
        for gi, g in enumerate(groups):
            i1g, i2g, b1g, b2g = [], [], [], []
            for j in g:
                t_new = c * per_core + j
                D = Ds[t_new]
                Dp = Ds_pos[j]
                Dgp = Dg[gi]
                it = plan["idx_tiles"][t_new]       # [P, D] new ids
                bt = plan["bias_tiles"][t_new]
                i1 = np.zeros((P, Dp), np.int32)
                i2 = np.zeros((P, Dgp), np.int32)
                b1 = np.full((P, Dp), NEG, np.float32)
                b2 = np.full((P, Dgp), NEG, np.float32)
                # flat-index gathers: pre-scale by the table row length
                i1[:, :D] = inv_rot[it] * (2 * D1)  # rotated ids (layer-1)
                # layer-2 flat index into the group-major AllGather layout
                tc_ = it // NOWN
                r_ = it % NOWN
                tj_ = r_ // P
                tgi_ = tj_ // GRP
                tjl_ = tj_ % GRP
                tp_ = r_ % P
                i2[:, :D] = (gbase[tgi_]
                             + (tc_ * glen[tgi_] + tjl_) * P * 2 * D2
                             + tp_ * 2 * D2)
                b1[:, :D] = bt
                b2[:, :D] = bt
                i1g.append(i1)
                i2g.append(i2)
                b1g.append(b1)
                b2g.append(b2)
            # group-major pack: [p, (tile, d)]
            idx1_list.append(np.concatenate(i1g, axis=1).reshape(-1))
            idx2_list.append(np.concatenate(i2g, axis=1).reshape(-1))
            b1c = np.concatenate(b1g, axis=1)
            b1_list.append((b1c + SHIFT).astype(np.float16).reshape(-1))
            b2_list.append(np.concatenate(b2g, axis=1)
                           .astype(np.float16).reshape(-1))
        in_maps.append(dict(
            xT=xT_c,
            w_kv1=w_kv1, w_qs1=w_qs1, w_2=w2,
            idx1_f=np.concatenate(idx1_list),
            idx2_f=np.concatenate(idx2_list),
            bias1_f=np.concatenate(b1_list),
            bias2_f=np.concatenate(b2_list),
        ))

    return nc, in_maps, plan


def kernel(**inputs):
    from concourse.bass_utils import run_bass_kernel_spmd

    nc, in_maps, plan = _prepare(inputs)
    res = run_bass_kernel_spmd(nc, in_maps, core_ids=list(range(N_CORES)))
    kernel.last_results = res

    out_new = np.concatenate([np.asarray(res.results[c]["out"])
                              for c in range(N_CORES)])
    mask = plan["perm"] < N
    out = np.empty((N, D2), np.float32)
    out[plan["perm"][mask]] = out_new[mask]
    return out
